# revision 30
# baseline (speedup 1.0000x reference)
"""Trainium2 Bass kernel for BinOverlapPredictionFromMaxProj (segment max + masked mean).

Full computation:
  ptm: (32, 8, 30, 1, 72, 72) f32, mem_mask: (32, 8, 30) bool
  n = 32*8 = 256 rows; per row: max over 5184-feature axis per mem (30), then
  masked mean over mems -> out (256,) f32.

Sharding: data-parallel over the 256 fused rows across 8 cores (32 rows each).
Per core: 960 segments x 5184 features (~19.9 MB) -> memory-bound.

Device plan per core: the shard's flat (960*5184,) stream is viewed as
(128 partitions, 15, 2592) with NO padding or host copy; each partition row
holds 7.5 segments, i.e. 15 aligned HALF-segments of 2592 floats. DMA needs
128 partitions for full rate (120 partitions measured 196 GB/s vs 420 GB/s).
  - NLOADS loads of (128, 15/NLOADS x 2592), contiguous 10368B*CPL per
    partition row, all queued on the gpsimd SWDGE queue.
  - vector.reduce_max(axis=X) -> stats[:, j]: max per half-segment.
  - one small SBUF->SBUF DMA repartitions the 1920 half-maxes from
    (128 partitions, 15) to (32 rows, 60) -- both APs walk half-segments in
    ascending order.
  - pairwise tensor_max joins the two halves of each segment -> (32, 30),
    then mask multiply, row-sum, reciprocal-count multiply -> out (32,).

The walrus codegen on this stack allows only ONE attached sync wait per
instruction, so TileContext's kernel-tail Drain (which waits on every
outstanding semaphore) is rewritten: waits beyond the first become standalone
wait_ge instructions. See _patch_tile_drain().
"""

import sys

import numpy as np

if "/opt/trn_rl_repo" not in sys.path:
    sys.path.insert(0, "/opt/trn_rl_repo")

NCORES = 8
NF, NS, NMEM, FEAT = 32, 8, 30, 5184
N = NF * NS  # 256
ROWS = N // NCORES  # 32 rows per core
SEGS = ROWS * NMEM  # 960 segments per core
PPART = 128  # partitions
HALF = FEAT // 2  # 2592 floats per half-segment
HPP = SEGS * 2 // PPART  # 15 half-segments per partition

_NC_CACHE = {}


def _patch_tile_drain():
    """Split the kernel-tail Drain's semaphore waits into standalone wait_ge
    instructions (one wait per instruction), to fit the walrus per-instruction
    sync-wait limit."""
    import concourse.tile as tile
    from concourse.vector_clock import ScopedClock

    if getattr(tile.TileContext._drain_and_barrier, "_single_wait_patch", False):
        return

    def _drain_and_barrier(self, tick_clock, wait_clock):
        drain_inst = self.nc.sync.drain()
        wait_clock.add_sem_waits(
            drain_inst.ins, ScopedClock({None: tick_clock.global_clock})
        )
        si = drain_inst.ins.sync_info
        waits = list(si.on_wait) if si is not None else []
        if len(waits) > 1:
            si.on_wait = [waits[0]]
            by_name = {h.name: h for h in self.sems.allocated().values()}
            for w in waits[1:]:
                self.nc.sync.wait_ge(by_name[w.ant_name], w.wait_value)

        self.nc.all_engine_barrier()
        assert self.sems is not None
        popped = self.nc._tile_sem_poison_stack.pop()
        assert popped is self._sem_poison
        self.nc.clear_and_free_semaphores(list(self.sems.allocated().values()))
        self.nc.all_engine_barrier()

    _drain_and_barrier._single_wait_patch = True
    tile.TileContext._drain_and_barrier = _drain_and_barrier


# LOAD_PLAN = half-segment columns per load (sums to HPP=15). Wide loads get
# full DMA rate (~423 GB/s measured; needs >=31KB partition rows).
LOAD_PLAN = (3, 3, 3, 3, 1, 1, 1)
LOAD_ENGINES = ("gpsimd",)
SMALL_ENGINE = "scalar"


def _build_nc(plan=None, load_engines=None, small_engine=None, cpl=None):
    import concourse.bass as bass
    import concourse.tile as tile
    from concourse import mybir

    _patch_tile_drain()

    if plan is None:
        plan = (cpl,) * (HPP // cpl) if cpl else LOAD_PLAN
    load_engines = LOAD_ENGINES if load_engines is None else load_engines
    small_engine = SMALL_ENGINE if small_engine is None else small_engine
    assert sum(plan) == HPP

    f32 = mybir.dt.float32
    X = mybir.AxisListType.X

    nc = bass.Bass("TRN2")
    ptm = nc.dram_tensor("ptm", [PPART, HPP, HALF], f32, kind="ExternalInput")
    maskf = nc.dram_tensor("maskf", [ROWS, NMEM], f32, kind="ExternalInput")
    out = nc.dram_tensor("out", [ROWS], f32, kind="ExternalOutput")

    small = getattr(nc, small_engine)

    with tile.TileContext(nc) as tc:
        with (
            tc.tile_pool(name="data", bufs=1) as dpool,
            tc.tile_pool(name="small", bufs=1) as spool,
        ):
            # Stage-2 mask prep runs early, overlapping the big loads.
            maskt = spool.tile([ROWS, NMEM], f32)
            small.dma_start(out=maskt[:], in_=maskf[:])
            cnt = spool.tile([ROWS, 1], f32)
            nc.vector.reduce_sum(out=cnt[:], in_=maskt[:], axis=X)
            rcnt = spool.tile([ROWS, 1], f32)
            nc.vector.reciprocal(out=rcnt[:], in_=cnt[:])

            # stats[p, j] = max of half-segment p*15 + j.
            stats = spool.tile([PPART, HPP], f32)
            col = 0
            for t, w in enumerate(plan):
                data = dpool.tile(
                    [PPART, w, HALF],
                    f32,
                    name=f"data{w}",
                    tag=f"data{w}",
                    bufs=plan.count(w),
                )
                eng = getattr(nc, load_engines[t % len(load_engines)])
                eng.dma_start(out=data[:], in_=ptm[:, col : col + w, :])
                nc.vector.reduce_max(out=stats[:, col : col + w], in_=data[:], axis=X)
                col += w

            # SBUF->SBUF repartition: both APs walk half-segments in ascending
            # order -> mx2[r, 2*m+h] == max of half h of segment r*30+m.
            # Split at column 12 so the bulk of the shuffle's latency hides
            # behind the final narrow loads' reduces.
            mx2 = spool.tile([ROWS, 2 * NMEM], f32)
            mx2g = mx2[:].rearrange("r (g c) -> r g c", g=4)
            small.dma_start(out=mx2g[:, :, 0:12], in_=stats[:, 0:12])
            small.dma_start(out=mx2g[:, :, 12:15], in_=stats[:, 12:15])
            # Tiny DVE read of the first shuffle's range so the pairmax only
            # needs one attached DMA wait (single-wait-per-instruction limit).
            sync1 = spool.tile([ROWS, 1], f32)
            nc.vector.tensor_copy(out=sync1[:], in_=mx2[:, 0:1])
            mx2v = mx2[:].rearrange("r (m two) -> r m two", two=2)
            mx = spool.tile([ROWS, NMEM], f32)
            nc.vector.tensor_max(out=mx[:], in0=mx2v[:, :, 0], in1=mx2v[:, :, 1])
            prod = spool.tile([ROWS, NMEM], f32)
            nc.vector.tensor_mul(out=prod[:], in0=mx[:], in1=maskt[:])
            msum = spool.tile([ROWS, 1], f32)
            nc.vector.reduce_sum(out=msum[:], in_=prod[:], axis=X)
            res = spool.tile([ROWS, 1], f32)
            nc.vector.tensor_mul(out=res[:], in0=msum[:], in1=rcnt[:])
            small.dma_start(out=out[:], in_=res[:, 0])

    return nc


def _get_nc():
    if "nc" not in _NC_CACHE:
        _NC_CACHE["nc"] = _build_nc()
    return _NC_CACHE["nc"]


def make_in_maps(ptm, mem_mask):
    ptm = np.ascontiguousarray(np.asarray(ptm, dtype=np.float32))
    mask = np.asarray(mem_mask)
    maskf = np.ascontiguousarray(mask.reshape(N, NMEM).astype(np.float32))
    ptm_flat = ptm.reshape(N * NMEM, FEAT)

    in_maps = []
    for i in range(NCORES):
        shard = ptm_flat[i * SEGS : (i + 1) * SEGS].reshape(PPART, HPP, HALF)
        in_maps.append(
            {"ptm": shard, "maskf": maskf[i * ROWS : (i + 1) * ROWS]}
        )
    return in_maps


def _ensure_ntff_hook():
    """Register the axon NTFF profiling hook (the container's antenv lacks
    axon_hooks; synthesize it from trn_agent_boot), and stub the artifact
    upload which has no bucket access here."""
    import types

    try:
        from antenv.axon_hooks import get_axon_ntff_profile_hook  # noqa: F401
    except ImportError:
        import antenv
        from trn_agent_boot.trn_boot import _ntff_profile_via_ctypes

        mod = types.ModuleType("antenv.axon_hooks")
        mod._hook = _ntff_profile_via_ctypes("/opt/axon/libaxon_pjrt.so")
        mod.set_axon_ntff_profile_hook = lambda h: setattr(mod, "_hook", h)
        mod.get_axon_ntff_profile_hook = lambda: mod._hook
        sys.modules["antenv.axon_hooks"] = mod
        antenv.axon_hooks = mod

    from concourse import bass_utils

    if not getattr(bass_utils.upload_artifacts, "_stubbed", False):
        def _no_upload(tmpdir):
            return str(tmpdir)

        _no_upload._stubbed = True
        bass_utils.upload_artifacts = _no_upload


def run(ptm, mem_mask, trace=False):
    from concourse.bass_utils import run_bass_kernel_spmd

    if trace:
        _ensure_ntff_hook()

    in_maps = make_in_maps(ptm, mem_mask)

    nc = _get_nc()
    kr = run_bass_kernel_spmd(nc, in_maps, list(range(NCORES)), trace=trace)
    out = np.concatenate([np.asarray(kr.results[i]["out"]) for i in range(NCORES)])
    return out.astype(np.float32), kr


def kernel(ptm, mem_mask):
    out, _ = run(ptm, mem_mask, trace=False)
    return out


# revision 31
# speedup vs baseline: 1.0050x; 1.0050x over previous
"""Trainium2 Bass kernel for BinOverlapPredictionFromMaxProj (segment max + masked mean).

Full computation:
  ptm: (32, 8, 30, 1, 72, 72) f32, mem_mask: (32, 8, 30) bool
  n = 32*8 = 256 rows; per row: max over 5184-feature axis per mem (30), then
  masked mean over mems -> out (256,) f32.

Sharding: data-parallel over the 256 fused rows across 8 cores (32 rows each).
Per core: 960 segments x 5184 features (~19.9 MB) -> memory-bound.

Device plan per core: the shard's flat (960*5184,) stream is viewed as
(128 partitions, 15, 2592) with NO padding or host copy; each partition row
holds 7.5 segments, i.e. 15 aligned HALF-segments of 2592 floats. DMA needs
128 partitions for full rate (120 partitions measured 196 GB/s vs 420 GB/s).
  - NLOADS loads of (128, 15/NLOADS x 2592), contiguous 10368B*CPL per
    partition row, all queued on the gpsimd SWDGE queue.
  - vector.reduce_max(axis=X) -> stats[:, j]: max per half-segment.
  - one small SBUF->SBUF DMA repartitions the 1920 half-maxes from
    (128 partitions, 15) to (32 rows, 60) -- both APs walk half-segments in
    ascending order.
  - pairwise tensor_max joins the two halves of each segment -> (32, 30),
    then mask multiply, row-sum, reciprocal-count multiply -> out (32,).

The walrus codegen on this stack allows only ONE attached sync wait per
instruction, so TileContext's kernel-tail Drain (which waits on every
outstanding semaphore) is rewritten: waits beyond the first become standalone
wait_ge instructions. See _patch_tile_drain().
"""

import sys

import numpy as np

if "/opt/trn_rl_repo" not in sys.path:
    sys.path.insert(0, "/opt/trn_rl_repo")

NCORES = 8
NF, NS, NMEM, FEAT = 32, 8, 30, 5184
N = NF * NS  # 256
ROWS = N // NCORES  # 32 rows per core
SEGS = ROWS * NMEM  # 960 segments per core
PPART = 128  # partitions
HALF = FEAT // 2  # 2592 floats per half-segment
HPP = SEGS * 2 // PPART  # 15 half-segments per partition

_NC_CACHE = {}


def _patch_tile_drain():
    """Split the kernel-tail Drain's semaphore waits into standalone wait_ge
    instructions (one wait per instruction), to fit the walrus per-instruction
    sync-wait limit."""
    import concourse.tile as tile
    from concourse.vector_clock import ScopedClock

    if getattr(tile.TileContext._drain_and_barrier, "_single_wait_patch", False):
        return

    def _drain_and_barrier(self, tick_clock, wait_clock):
        drain_inst = self.nc.sync.drain()
        wait_clock.add_sem_waits(
            drain_inst.ins, ScopedClock({None: tick_clock.global_clock})
        )
        si = drain_inst.ins.sync_info
        waits = list(si.on_wait) if si is not None else []
        if len(waits) > 1:
            si.on_wait = [waits[0]]
            by_name = {h.name: h for h in self.sems.allocated().values()}
            for w in waits[1:]:
                self.nc.sync.wait_ge(by_name[w.ant_name], w.wait_value)

        self.nc.all_engine_barrier()
        assert self.sems is not None
        popped = self.nc._tile_sem_poison_stack.pop()
        assert popped is self._sem_poison
        self.nc.clear_and_free_semaphores(list(self.sems.allocated().values()))
        self.nc.all_engine_barrier()

    _drain_and_barrier._single_wait_patch = True
    tile.TileContext._drain_and_barrier = _drain_and_barrier


# LOAD_PLAN = half-segment columns per load (sums to HPP=15). Wide loads get
# full DMA rate (~423 GB/s measured; needs >=31KB partition rows).
LOAD_PLAN = (3, 3, 3, 3, 1, 1, 1)
LOAD_ENGINES = ("gpsimd",)
SMALL_ENGINE = "scalar"


def _build_nc(plan=None, load_engines=None, small_engine=None, cpl=None):
    import concourse.bass as bass
    import concourse.tile as tile
    from concourse import mybir

    _patch_tile_drain()

    if plan is None:
        plan = (cpl,) * (HPP // cpl) if cpl else LOAD_PLAN
    load_engines = LOAD_ENGINES if load_engines is None else load_engines
    small_engine = SMALL_ENGINE if small_engine is None else small_engine
    assert sum(plan) == HPP

    f32 = mybir.dt.float32
    X = mybir.AxisListType.X

    nc = bass.Bass("TRN2")
    ptm = nc.dram_tensor("ptm", [PPART, HPP, HALF], f32, kind="ExternalInput")
    maskf = nc.dram_tensor("maskf", [ROWS, NMEM], f32, kind="ExternalInput")
    out = nc.dram_tensor("out", [ROWS], f32, kind="ExternalOutput")

    small = getattr(nc, small_engine)

    with tile.TileContext(nc) as tc:
        with (
            tc.tile_pool(name="data", bufs=1) as dpool,
            tc.tile_pool(name="small", bufs=1) as spool,
        ):
            # Stage-2 mask prep runs early, overlapping the big loads.
            maskt = spool.tile([ROWS, NMEM], f32)
            small.dma_start(out=maskt[:], in_=maskf[:])
            cnt = spool.tile([ROWS, 1], f32)
            nc.vector.reduce_sum(out=cnt[:], in_=maskt[:], axis=X)
            rcnt = spool.tile([ROWS, 1], f32)
            nc.vector.reciprocal(out=rcnt[:], in_=cnt[:])

            # stats[p, j] = max of half-segment p*15 + j.
            stats = spool.tile([PPART, HPP], f32)
            col = 0
            for t, w in enumerate(plan):
                data = dpool.tile(
                    [PPART, w, HALF],
                    f32,
                    name=f"data{w}",
                    tag=f"data{w}",
                    bufs=plan.count(w),
                )
                eng = getattr(nc, load_engines[t % len(load_engines)])
                eng.dma_start(out=data[:], in_=ptm[:, col : col + w, :])
                nc.vector.reduce_max(out=stats[:, col : col + w], in_=data[:], axis=X)
                col += w

            # SBUF->SBUF repartition: both APs walk half-segments in ascending
            # order -> mx2[r, 2*m+h] == max of half h of segment r*30+m.
            mx2 = spool.tile([ROWS, 2 * NMEM], f32)
            small.dma_start(out=mx2[:], in_=stats[:])
            mx2v = mx2[:].rearrange("r (m two) -> r m two", two=2)
            mx = spool.tile([ROWS, NMEM], f32)
            nc.vector.tensor_max(out=mx[:], in0=mx2v[:, :, 0], in1=mx2v[:, :, 1])
            prod = spool.tile([ROWS, NMEM], f32)
            nc.vector.tensor_mul(out=prod[:], in0=mx[:], in1=maskt[:])
            msum = spool.tile([ROWS, 1], f32)
            nc.vector.reduce_sum(out=msum[:], in_=prod[:], axis=X)
            res = spool.tile([ROWS, 1], f32)
            nc.vector.tensor_mul(out=res[:], in0=msum[:], in1=rcnt[:])
            small.dma_start(out=out[:], in_=res[:, 0])

    return nc


def _get_nc():
    if "nc" not in _NC_CACHE:
        _NC_CACHE["nc"] = _build_nc()
    return _NC_CACHE["nc"]


def make_in_maps(ptm, mem_mask):
    ptm = np.ascontiguousarray(np.asarray(ptm, dtype=np.float32))
    mask = np.asarray(mem_mask)
    maskf = np.ascontiguousarray(mask.reshape(N, NMEM).astype(np.float32))
    ptm_flat = ptm.reshape(N * NMEM, FEAT)

    in_maps = []
    for i in range(NCORES):
        shard = ptm_flat[i * SEGS : (i + 1) * SEGS].reshape(PPART, HPP, HALF)
        in_maps.append(
            {"ptm": shard, "maskf": maskf[i * ROWS : (i + 1) * ROWS]}
        )
    return in_maps


def _ensure_ntff_hook():
    """Register the axon NTFF profiling hook (the container's antenv lacks
    axon_hooks; synthesize it from trn_agent_boot), and stub the artifact
    upload which has no bucket access here."""
    import types

    try:
        from antenv.axon_hooks import get_axon_ntff_profile_hook  # noqa: F401
    except ImportError:
        import antenv
        from trn_agent_boot.trn_boot import _ntff_profile_via_ctypes

        mod = types.ModuleType("antenv.axon_hooks")
        mod._hook = _ntff_profile_via_ctypes("/opt/axon/libaxon_pjrt.so")
        mod.set_axon_ntff_profile_hook = lambda h: setattr(mod, "_hook", h)
        mod.get_axon_ntff_profile_hook = lambda: mod._hook
        sys.modules["antenv.axon_hooks"] = mod
        antenv.axon_hooks = mod

    from concourse import bass_utils

    if not getattr(bass_utils.upload_artifacts, "_stubbed", False):
        def _no_upload(tmpdir):
            return str(tmpdir)

        _no_upload._stubbed = True
        bass_utils.upload_artifacts = _no_upload


def run(ptm, mem_mask, trace=False):
    from concourse.bass_utils import run_bass_kernel_spmd

    if trace:
        _ensure_ntff_hook()

    in_maps = make_in_maps(ptm, mem_mask)

    nc = _get_nc()
    kr = run_bass_kernel_spmd(nc, in_maps, list(range(NCORES)), trace=trace)
    out = np.concatenate([np.asarray(kr.results[i]["out"]) for i in range(NCORES)])
    return out.astype(np.float32), kr


def kernel(ptm, mem_mask):
    out, _ = run(ptm, mem_mask, trace=False)
    return out
